# revision 14
# baseline (speedup 1.0000x reference)
"""Tensor-parallel GQA multi-head attention (RoPE + causal softmax) for 8 trn2 cores.

Sharding: 8 cores = 2 batches x 4 head-groups. Core c handles batch c//4 and
q-heads [8g, 8g+8) / kv-heads {2g, 2g+1} where g = c%4. Each core projects its
batch's tokens with its weight shard, runs flash-style causal attention in
transposed (feature-major) layout, applies the output projection, and the four
cores of a batch AllReduce the partial [S, D] output. Host stacks batch 0/1.
"""

import sys

sys.path.insert(0, "/opt/trn_rl_repo")

import numpy as np

import concourse.bass as bass
import concourse.bacc as bacc
import concourse.mybir as mybir
from concourse import tile
from concourse.bass_utils import run_bass_kernel_spmd

B, S, D = 2, 2048, 2048
N_HEADS, N_KV, HD = 32, 8, 64
NCORES = 8
NG = 4  # head groups = cores per batch
QH = 8  # q-heads per core
KVH = 2  # kv-heads per core
FQ = QH * HD  # 512
FKV = 2 * KVH * HD  # 256 (K then V)
SCALE = 1.0 / 8.0  # 1/sqrt(HD)
MASK_NEG = -30000.0

QTILE = 512
KTILE = 128
NSLAB = S // QTILE  # 4
ND = D // 128  # 16 contraction chunks
NKT = S // KTILE  # 16

F32 = mybir.dt.float32
F32R = mybir.dt.float32r
EXP = mybir.ActivationFunctionType.Exp

# matmul operand cast: float32r streams fp32 data at full PE rate (N>=256)
MM = F32R


def _c(ap):
    return ap  # operands are float32r-native


def _build_kernel(tc, io):
    nc = tc.nc
    xT, wq, wkv, wo = io["xT"], io["wq"], io["wkv"], io["wo"]
    cos2, sin2s, trimask, sel = io["cos2"], io["sin2s"], io["trimask"], io["sel"]
    out_full = io["out"]

    # ---- pools with explicit lifetimes (per-partition SBUF is tight) ----
    const = tc.alloc_tile_pool(name="const", bufs=1)          # whole kernel
    dram = tc.alloc_tile_pool(name="dram", bufs=1, space="DRAM")
    qkv = tc.alloc_tile_pool(name="qkv", bufs=1)              # A..D
    vvp = tc.alloc_tile_pool(name="vvp", bufs=1, side="right")   # A..C
    tables = tc.alloc_tile_pool(name="tables", bufs=1, side="right")  # A..B

    trimask_t = const.tile([KTILE, KTILE], F32)
    nc.sync.dma_start(trimask_t[:], trimask[:])
    sel_t = const.tile([QH, FQ], F32)
    nc.sync.dma_start(sel_t[:], sel[:])
    ident = const.tile([128, 64], F32)
    nc.gpsimd.memset(ident[:], 0.0)
    for p in (0, 64):
        nc.gpsimd.affine_select(
            out=ident[p:p + 64, :], in_=ident[p:p + 64, :],
            compare_op=mybir.AluOpType.not_equal,
            fill=1.0, base=0, pattern=[[-1, 64]], channel_multiplier=1,
        )

    ones_col = const.tile([128, 1], F32)
    nc.vector.memset(ones_col[:], 1.0)

    cos2_t = tables.tile([128, S], F32)
    nc.sync.dma_start(cos2_t[:], cos2[:])
    sin2s_t = tables.tile([128, S], F32)
    nc.sync.dma_start(sin2s_t[:], sin2s[:])

    QT = [qkv.tile([128, S], F32R, name=f"qt{t}") for t in range(4)]
    KK = qkv.tile([128, S], F32R)  # rows 0:64 K^T kv0, 64:128 K^T kv1
    VV = vvp.tile([128, S], F32)  # same layout, V^T (no rope)

    # ---------------- phase A: QKV projections ----------------
    wA = tc.alloc_tile_pool(name="wA", bufs=1)
    xq_pool = tc.alloc_tile_pool(name="xq", bufs=2)
    psA = tc.alloc_tile_pool(name="psA", bufs=4, space="PSUM")
    Wt = {}
    for f in range(6):
        for k in range(ND):
            w = wA.tile([128, 128], F32R, name=f"w{f}_{k}")
            if f < 4:
                src = wq[k * 128:(k + 1) * 128, f * 128:(f + 1) * 128]
            else:
                src = wkv[k * 128:(k + 1) * 128, (f - 4) * 128:(f - 3) * 128]
            nc.sync.dma_start(w[:], src)
            Wt[f, k] = w
    dests = QT + [KK, VV]
    for j in range(NSLAB):
        xts = []
        for k in range(ND):
            xt = xq_pool.tile([128, QTILE], F32R, name="xt", tag=f"xt{k}")
            nc.sync.dma_start(
                xt[:], xT[k * 128:(k + 1) * 128, j * QTILE:(j + 1) * QTILE])
            xts.append(xt)
        for f in range(6):
            ps = psA.tile([128, QTILE], F32, name="psA", tag="psA")
            for k in range(ND):
                nc.tensor.matmul(ps[:], _c(Wt[f, k][:]), _c(xts[k][:]),
                                 start=(k == 0), stop=(k == ND - 1))
            nc.scalar.copy(dests[f][:, j * QTILE:(j + 1) * QTILE], ps[:])
    psA.release()
    xq_pool.release()
    wA.release()  # LIFO: xq then wA on the left stack

    # ---------------- phase B: RoPE on QT (4 tiles) and KK ----------------
    rp = tc.alloc_tile_pool(name="rope", bufs=2)
    for t in range(5):
        src = QT[t] if t < 4 else KK
        qsw = rp.tile([128, S], F32R, name="qsw", tag="qsw")
        # swapped halves per 64-row head block (sign baked into sin2s)
        for p in (0, 64):
            nc.sync.dma_start(qsw[p:p + 32, :], src[p + 32:p + 64, :])
            nc.sync.dma_start(qsw[p + 32:p + 64, :], src[p:p + 32, :])
        t1 = rp.tile([128, S], F32, name="t1", tag="t1")
        nc.vector.tensor_mul(t1[:], src[:], cos2_t[:])
        t2 = rp.tile([128, S], F32, name="t2", tag="t2")
        nc.vector.tensor_mul(t2[:], qsw[:], sin2s_t[:])
        nc.vector.tensor_add(src[:], t1[:], t2[:])
    rp.release()
    tables.release()

    # ---------------- phase C: transpose V -> [k, d] chunks + ones col -----
    vap = tc.alloc_tile_pool(name="vap", bufs=1)              # C..D (left)
    psC = tc.alloc_tile_pool(name="psC", bufs=2, space="PSUM")
    VA = {}
    for kv in range(KVH):
        for i in range(NKT):
            tp = psC.tile([128, HD], F32, name="tp", tag="tp")
            nc.tensor.matmul(tp[:], VV[kv * 64:(kv + 1) * 64,
                                        i * 128:(i + 1) * 128],
                             ident[kv * 64:(kv + 1) * 64, :],
                             is_transpose=True, start=True, stop=True)
            va = vap.tile([128, HD + 1], F32R, name=f"va{kv}_{i}")
            nc.scalar.copy(va[:, 0:HD], tp[:])
            nc.scalar.copy(va[:, HD:HD + 1], ones_col[:])
            VA[kv, i] = va
    psC.release()
    vvp.release()

    # preload wo while attention runs
    wop = tc.alloc_tile_pool(name="wop", bufs=1, side="right")   # C..F
    WO = {}
    for fc in range(4):
        for dn in range(4):
            w = wop.tile([128, QTILE], F32R, name=f"wo{fc}_{dn}")
            nc.sync.dma_start(
                w[:], wo[fc * 128:(fc + 1) * 128, dn * QTILE:(dn + 1) * QTILE])
            WO[fc, dn] = w

    # ---------------- phase D: causal attention ----------------
    aop = tc.alloc_tile_pool(name="aop", bufs=1, side="right")   # D..F
    AO = [aop.tile([128, S], F32R, name=f"ao{t}") for t in range(4)]
    denom = aop.tile([QH, S], F32R)
    denomR = aop.tile([QH, S], F32)

    psS = tc.alloc_tile_pool(name="psS", bufs=2, space="PSUM")
    psO = tc.alloc_tile_pool(name="psO", bufs=2, space="PSUM")
    pexp = tc.alloc_tile_pool(name="pexp", bufs=4)
    evac = tc.alloc_tile_pool(name="evac", bufs=4)
    for t in range(4):
        for j in range(NSLAB):
            qs = slice(j * QTILE, (j + 1) * QTILE)
            oA = psO.tile([HD + 1, QTILE], F32, name="oA", tag="oA")
            oB = psO.tile([HD + 1, QTILE], F32, name="oB", tag="oB")
            nkt = 4 * j + 4
            for i in range(nkt):
                r = i - 4 * j
                off = max(r, 0) * KTILE
                ks = slice(i * KTILE, (i + 1) * KTILE)
                qv = slice(j * QTILE + off, (j + 1) * QTILE)
                sA = psS.tile([KTILE, QTILE], F32, name="sA", tag="sA")
                sB = psS.tile([KTILE, QTILE], F32, name="sB", tag="sB")
                nc.tensor.matmul(sA[:, off:], _c(KK[0:64, ks]),
                                 _c(QT[t][0:64, qv]), start=True, stop=True)
                nc.tensor.matmul(sB[:, off:], _c(KK[64:128, ks]),
                                 _c(QT[t][64:128, qv]), start=True, stop=True)
                if r >= 0:
                    nc.vector.tensor_add(sA[:, off:off + KTILE],
                                         sA[:, off:off + KTILE], trimask_t[:])
                    nc.vector.tensor_add(sB[:, off:off + KTILE],
                                         sB[:, off:off + KTILE], trimask_t[:])
                pA = pexp.tile([KTILE, QTILE], F32R, name="pA", tag="pA")
                pB = pexp.tile([KTILE, QTILE], F32R, name="pB", tag="pB")
                nc.scalar.activation(pA[:, off:], sA[:, off:], EXP, scale=SCALE)
                nc.scalar.activation(pB[:, off:], sB[:, off:], EXP, scale=SCALE)
                nc.tensor.matmul(oA[:, off:], _c(VA[0, i][:]), _c(pA[:, off:]),
                                 start=(i == 0), stop=(i == nkt - 1))
                nc.tensor.matmul(oB[:, off:], _c(VA[1, i][:]), _c(pB[:, off:]),
                                 start=(i == 0), stop=(i == nkt - 1))
            # evacuate: rows 0:64 outT, row 64 denominator
            tA = evac.tile([HD + 1, QTILE], F32R, name="tA", tag="tA")
            tB = evac.tile([HD + 1, QTILE], F32R, name="tB", tag="tB")
            nc.scalar.copy(tA[:], oA[:])
            nc.scalar.copy(tB[:], oB[:])
            nc.sync.dma_start(AO[t][0:64, qs], tA[0:64, :])
            nc.sync.dma_start(AO[t][64:128, qs], tB[0:64, :])
            nc.sync.dma_start(denom[t:t + 1, qs], tA[64:65, :])
            nc.sync.dma_start(denom[t + 4:t + 5, qs], tB[64:65, :])
    psO.release()
    psS.release()
    evac.release()
    pexp.release()
    vap.release()
    qkv.release()

    # ---------------- phase E: normalize by softmax denominator ------------
    nc.vector.reciprocal(denomR[:], denom[:])
    psBC = tc.alloc_tile_pool(name="psBC", bufs=4, space="PSUM")
    for t in range(4):
        for j in range(NSLAB):
            qs = slice(j * QTILE, (j + 1) * QTILE)
            bc = psBC.tile([128, QTILE], F32, name="bc", tag="bc")
            nc.tensor.matmul(bc[:], sel_t[:, t * 128:(t + 1) * 128],
                             denomR[:, qs], start=True, stop=True)
            nc.vector.tensor_mul(AO[t][:, qs], AO[t][:, qs], bc[:])
    psBC.release()

    # ---------------- phase F: output projection + AllReduce ---------------
    partial = dram.tile([S, D], F32)
    ar_out = dram.tile([S, D], F32)
    psW = tc.alloc_tile_pool(name="psW", bufs=4, space="PSUM")
    ostg = tc.alloc_tile_pool(name="ostg", bufs=4)
    for jq in range(NKT):  # 16 q-tiles of 128
        qsl = slice(jq * 128, (jq + 1) * 128)
        for dn in range(4):
            ps = psW.tile([128, QTILE], F32, name="psW", tag="psW")
            for fc in range(4):
                nc.tensor.matmul(ps[:], _c(AO[fc][:, qsl]), _c(WO[fc, dn][:]),
                                 start=(fc == 0), stop=(fc == 3))
            og = ostg.tile([128, QTILE], F32, name="og", tag="og")
            nc.scalar.copy(og[:], ps[:])
            nc.sync.dma_start(
                partial[jq * 128:(jq + 1) * 128,
                        dn * QTILE:(dn + 1) * QTILE], og[:])
    psW.release()
    ostg.release()
    aop.release()
    wop.release()

    if io.get("single"):
        nc.sync.dma_start(out_full[:], partial[:])
    else:
        nc.gpsimd.collective_compute(
            "AllReduce",
            mybir.AluOpType.add,
            replica_groups=[[0, 1, 2, 3], [4, 5, 6, 7]],
            ins=[partial[:]],
            outs=[ar_out[:]],
        )
        nc.sync.dma_start(out_full[:], ar_out[:])
    dram.release()
    const.release()


def _build(single=False):
    nc = bacc.Bacc("TRN2", target_bir_lowering=False, debug=False,
                   num_devices=1 if single else NCORES)
    io = {
        "xT": nc.dram_tensor("xT", [D, S], F32R, kind="ExternalInput").ap(),
        "wq": nc.dram_tensor("wq", [D, FQ], F32R, kind="ExternalInput").ap(),
        "wkv": nc.dram_tensor("wkv", [D, FKV], F32R, kind="ExternalInput").ap(),
        "wo": nc.dram_tensor("wo", [FQ, D], F32R, kind="ExternalInput").ap(),
        "cos2": nc.dram_tensor("cos2", [128, S], F32, kind="ExternalInput").ap(),
        "sin2s": nc.dram_tensor("sin2s", [128, S], F32, kind="ExternalInput").ap(),
        "trimask": nc.dram_tensor("trimask", [KTILE, KTILE], F32,
                                  kind="ExternalInput").ap(),
        "sel": nc.dram_tensor("sel", [QH, FQ], F32, kind="ExternalInput").ap(),
        "out": nc.dram_tensor("out", [S, D], F32, kind="ExternalOutput").ap(),
    }
    io["single"] = single
    with tile.TileContext(nc) as tc:
        _build_kernel(tc, io)
    nc.compile()
    return nc


_CACHE = {}


def _get_program():
    if "nc" not in _CACHE:
        _CACHE["nc"] = _build()
    return _CACHE["nc"]


def _host_inputs(x, wq, wk, wv, wo):
    x = np.ascontiguousarray(x, np.float32)
    inv = 1.0 / (10000.0 ** (np.arange(0, HD, 2, dtype=np.float64) / HD))
    pos = np.arange(S, dtype=np.float64)
    freqs = np.outer(pos, inv)  # [S, 32]
    emb = np.concatenate([freqs, freqs], axis=1)  # [S, 64]
    cos = np.cos(emb).T.astype(np.float32)  # [64, S]
    sin = np.sin(emb).T.astype(np.float32)
    cos2 = np.concatenate([cos, cos], axis=0)  # [128, S]
    sin2s = np.concatenate([-sin[:32], sin[32:], -sin[:32], sin[32:]], axis=0)

    kk, qq = np.meshgrid(np.arange(KTILE), np.arange(KTILE), indexing="ij")
    trimask = np.where(kk <= qq, 0.0, MASK_NEG).astype(np.float32)

    # attn_outT row layout per pair-tile t: rows 0:64 head t, 64:128 head t+4
    sel = np.zeros((QH, FQ), np.float32)
    for t in range(4):
        sel[t, t * 128:t * 128 + 64] = 1.0
        sel[t + 4, t * 128 + 64:(t + 1) * 128] = 1.0

    xT = [np.ascontiguousarray(x[b].T) for b in range(B)]
    in_maps = []
    for c in range(NCORES):
        b, g = c // NG, c % NG
        # pair-tile column order: heads (t, t+4) interleaved per 128-col tile
        qcols = []
        wrows = []
        for t in range(4):
            for h in (8 * g + t, 8 * g + t + 4):
                qcols.append(wq[:, h * HD:(h + 1) * HD])
                wrows.append(wo[h * HD:(h + 1) * HD, :])
        wq_p = np.ascontiguousarray(np.concatenate(qcols, axis=1), np.float32)
        wo_p = np.ascontiguousarray(np.concatenate(wrows, axis=0), np.float32)
        kv0 = 2 * g
        wkv_p = np.ascontiguousarray(np.concatenate(
            [wk[:, kv0 * HD:(kv0 + 2) * HD], wv[:, kv0 * HD:(kv0 + 2) * HD]],
            axis=1), np.float32)
        in_maps.append({
            "xT": xT[b], "wq": wq_p, "wkv": wkv_p, "wo": wo_p,
            "cos2": cos2, "sin2s": sin2s, "trimask": trimask, "sel": sel,
        })
    return in_maps


def run(x, wq, wk, wv, wo, trace=False, **trace_kwargs):
    nc = _get_program()
    in_maps = _host_inputs(x, wq, wk, wv, wo)
    res = run_bass_kernel_spmd(nc, in_maps, list(range(NCORES)),
                               trace=trace, **trace_kwargs)
    out = np.stack([res.results[0]["out"], res.results[4]["out"]], axis=0)
    return out, res


def kernel(x, wq, wk, wv, wo):
    out, _ = run(x, wq, wk, wv, wo)
    return out.astype(np.float32)


# revision 16
# speedup vs baseline: 1.5934x; 1.5934x over previous
"""Tensor-parallel GQA multi-head attention (RoPE + causal softmax) for 8 trn2 cores.

Sharding: 8 cores = 2 batches x 4 head-groups. Core c handles batch c//4 and
q-heads [8g, 8g+8) / kv-heads {2g, 2g+1} where g = c%4. Each core projects its
batch's tokens with its weight shard, runs flash-style causal attention in
transposed (feature-major) layout, applies the output projection, and the four
cores of a batch AllReduce the partial [S, D] output. Host stacks batch 0/1.
"""

import sys

sys.path.insert(0, "/opt/trn_rl_repo")

import numpy as np

import concourse.bass as bass
import concourse.bacc as bacc
import concourse.mybir as mybir
from concourse import tile
from concourse.bass_utils import run_bass_kernel_spmd

B, S, D = 2, 2048, 2048
N_HEADS, N_KV, HD = 32, 8, 64
NCORES = 8
NG = 4  # head groups = cores per batch
QH = 8  # q-heads per core
KVH = 2  # kv-heads per core
FQ = QH * HD  # 512
FKV = 2 * KVH * HD  # 256 (K then V)
SCALE = 1.0 / 8.0  # 1/sqrt(HD)
MASK_NEG = -30000.0

QTILE = 512
KTILE = 128
NSLAB = S // QTILE  # 4
ND = D // 128  # 16 contraction chunks
NKT = S // KTILE  # 16

F32 = mybir.dt.float32
F32R = mybir.dt.float32r
EXP = mybir.ActivationFunctionType.Exp

# matmul operand cast: float32r streams fp32 data at full PE rate (N>=256)
MM = F32R


def _c(ap):
    return ap  # operands are float32r-native


def _build_kernel(tc, io):
    nc = tc.nc
    xT, wq, wkv, wo = io["xT"], io["wq"], io["wkv"], io["wo"]
    cos2, sin2s, trimask, sel = io["cos2"], io["sin2s"], io["trimask"], io["sel"]
    out_full = io["out"]

    # ---- pools with explicit lifetimes (per-partition SBUF is tight) ----
    const = tc.alloc_tile_pool(name="const", bufs=1)          # whole kernel
    dram = tc.alloc_tile_pool(name="dram", bufs=1, space="DRAM")
    qkv = tc.alloc_tile_pool(name="qkv", bufs=1)              # A..D
    vvp = tc.alloc_tile_pool(name="vvp", bufs=1, side="right")   # A..C
    tables = tc.alloc_tile_pool(name="tables", bufs=1, side="right")  # A..B

    trimask_t = const.tile([KTILE, KTILE], F32)
    nc.sync.dma_start(trimask_t[:], trimask[:])
    sel_t = const.tile([QH, FQ], F32)
    nc.sync.dma_start(sel_t[:], sel[:])
    ident = const.tile([128, 64], F32)
    nc.gpsimd.memset(ident[:], 0.0)
    for p in (0, 64):
        nc.gpsimd.affine_select(
            out=ident[p:p + 64, :], in_=ident[p:p + 64, :],
            compare_op=mybir.AluOpType.not_equal,
            fill=1.0, base=0, pattern=[[-1, 64]], channel_multiplier=1,
        )

    ones_col = const.tile([128, 1], F32)
    nc.vector.memset(ones_col[:], 1.0)

    cos2_t = tables.tile([128, S], F32)
    nc.sync.dma_start(cos2_t[:], cos2[:])
    sin2s_t = tables.tile([128, S], F32)
    nc.sync.dma_start(sin2s_t[:], sin2s[:])

    QT = [qkv.tile([128, S], F32R, name=f"qt{t}") for t in range(4)]
    KK = qkv.tile([128, S], F32R)  # rows 0:64 K^T kv0, 64:128 K^T kv1
    VV = vvp.tile([128, S], F32)  # same layout, V^T (no rope)

    # ---------------- phase A: QKV projections ----------------
    wA = tc.alloc_tile_pool(name="wA", bufs=1)
    xq_pool = tc.alloc_tile_pool(name="xq", bufs=2)
    psA = tc.alloc_tile_pool(name="psA", bufs=4, space="PSUM")
    def load_xslab(j):
        xts = []
        for k in range(ND):
            xt = xq_pool.tile([128, QTILE], F32R, name="xt", tag=f"xt{k}")
            nc.sync.dma_start(
                xt[:], xT[k * 128:(k + 1) * 128, j * QTILE:(j + 1) * QTILE])
            xts.append(xt)
        return xts

    xts = load_xslab(0)  # first activations slab before the weight bulk
    Wt = {}
    for f in range(6):
        for k in range(ND):
            w = wA.tile([128, 128], F32R, name=f"w{f}_{k}")
            if f < 4:
                src = wq[k * 128:(k + 1) * 128, f * 128:(f + 1) * 128]
            else:
                src = wkv[k * 128:(k + 1) * 128, (f - 4) * 128:(f - 3) * 128]
            nc.sync.dma_start(w[:], src)
            Wt[f, k] = w
    dests = QT + [KK, VV]
    for j in range(NSLAB):
        if j > 0:
            xts = load_xslab(j)
        for f in range(6):
            ps = psA.tile([128, QTILE], F32, name="psA", tag="psA")
            for k in range(ND):
                nc.tensor.matmul(ps[:], _c(Wt[f, k][:]), _c(xts[k][:]),
                                 start=(k == 0), stop=(k == ND - 1))
            nc.scalar.copy(dests[f][:, j * QTILE:(j + 1) * QTILE], ps[:])
    psA.release()
    xq_pool.release()
    wA.release()  # LIFO: xq then wA on the left stack

    # ---------------- phase B: RoPE on QT (4 tiles) and KK ----------------
    rp = tc.alloc_tile_pool(name="rope", bufs=2)
    for t in range(5):
        src = QT[t] if t < 4 else KK
        qsw = rp.tile([128, S], F32R, name="qsw", tag="qsw")
        # swapped halves per 64-row head block (sign baked into sin2s)
        for p in (0, 64):
            nc.sync.dma_start(qsw[p:p + 32, :], src[p + 32:p + 64, :])
            nc.sync.dma_start(qsw[p + 32:p + 64, :], src[p:p + 32, :])
        t1 = rp.tile([128, S], F32, name="t1", tag="t1")
        nc.vector.tensor_mul(t1[:], src[:], cos2_t[:])
        t2 = rp.tile([128, S], F32, name="t2", tag="t2")
        nc.vector.tensor_mul(t2[:], qsw[:], sin2s_t[:])
        nc.vector.tensor_add(src[:], t1[:], t2[:])
    rp.release()
    tables.release()

    # ---------------- phase C: transpose V -> [k, d] chunks + ones col -----
    vap = tc.alloc_tile_pool(name="vap", bufs=1)              # C..D (left)
    psC = tc.alloc_tile_pool(name="psC", bufs=2, space="PSUM")
    VA = {}
    for kv in range(KVH):
        for i in range(NKT):
            tp = psC.tile([128, HD], F32, name="tp", tag="tp")
            nc.tensor.matmul(tp[:], VV[kv * 64:(kv + 1) * 64,
                                        i * 128:(i + 1) * 128],
                             ident[kv * 64:(kv + 1) * 64, :],
                             is_transpose=True, start=True, stop=True)
            va = vap.tile([128, HD + 1], F32R, name=f"va{kv}_{i}")
            nc.scalar.copy(va[:, 0:HD], tp[:])
            nc.scalar.copy(va[:, HD:HD + 1], ones_col[:])
            VA[kv, i] = va
    psC.release()
    vvp.release()

    # preload wo while attention runs
    wop = tc.alloc_tile_pool(name="wop", bufs=1, side="right")   # C..F
    WO = {}
    for fc in range(4):
        for dn in range(4):
            w = wop.tile([128, QTILE], F32R, name=f"wo{fc}_{dn}")
            nc.sync.dma_start(
                w[:], wo[fc * 128:(fc + 1) * 128, dn * QTILE:(dn + 1) * QTILE])
            WO[fc, dn] = w

    # ---------------- phases D-F merged, pipelined per q-slab --------------
    aop = tc.alloc_tile_pool(name="aop", bufs=1, side="right")   # D..F
    AO = [aop.tile([128, S], F32R, name=f"ao{t}") for t in range(4)]
    denom = aop.tile([QH, S], F32R)
    denomR = aop.tile([QH, S], F32)

    partial = dram.tile([S, D], F32)
    rs_out = [dram.tile([128, D], F32, name=f"rs{j}") for j in range(NSLAB)]

    psS = tc.alloc_tile_pool(name="psS", bufs=2, space="PSUM")
    psW = tc.alloc_tile_pool(name="psW", bufs=2, space="PSUM")
    pexp = tc.alloc_tile_pool(name="pexp", bufs=4)
    evac = tc.alloc_tile_pool(name="evac", bufs=4)
    for j in range(NSLAB):
        qs = slice(j * QTILE, (j + 1) * QTILE)
        # -- attention for all head pairs on this q-slab --
        for t in range(4):
            oA = psS.tile([HD + 1, QTILE], F32, name="oA", tag="o")
            oB = psS.tile([HD + 1, QTILE], F32, name="oB", tag="o")
            nkt = 4 * j + 4
            for i in range(nkt):
                r = i - 4 * j
                off = max(r, 0) * KTILE
                ks = slice(i * KTILE, (i + 1) * KTILE)
                qv = slice(j * QTILE + off, (j + 1) * QTILE)
                sA = psS.tile([KTILE, QTILE], F32, name="sA", tag="sA")
                sB = psS.tile([KTILE, QTILE], F32, name="sB", tag="sB")
                nc.tensor.matmul(sA[:, off:], _c(KK[0:64, ks]),
                                 _c(QT[t][0:64, qv]), start=True, stop=True)
                nc.tensor.matmul(sB[:, off:], _c(KK[64:128, ks]),
                                 _c(QT[t][64:128, qv]), start=True, stop=True)
                if r >= 0:
                    nc.vector.tensor_add(sA[:, off:off + KTILE],
                                         sA[:, off:off + KTILE], trimask_t[:])
                    nc.vector.tensor_add(sB[:, off:off + KTILE],
                                         sB[:, off:off + KTILE], trimask_t[:])
                pA = pexp.tile([KTILE, QTILE], F32R, name="pA", tag="pA")
                pB = pexp.tile([KTILE, QTILE], F32R, name="pB", tag="pB")
                nc.scalar.activation(pA[:, off:], sA[:, off:], EXP, scale=SCALE)
                nc.scalar.activation(pB[:, off:], sB[:, off:], EXP, scale=SCALE)
                nc.tensor.matmul(oA[:, off:], _c(VA[0, i][:]), _c(pA[:, off:]),
                                 start=(i == 0), stop=(i == nkt - 1))
                nc.tensor.matmul(oB[:, off:], _c(VA[1, i][:]), _c(pB[:, off:]),
                                 start=(i == 0), stop=(i == nkt - 1))
            # evacuate: rows 0:64 outT, row 64 denominator
            tA = evac.tile([HD + 1, QTILE], F32R, name="tA", tag="tA")
            tB = evac.tile([HD + 1, QTILE], F32R, name="tB", tag="tB")
            nc.vector.tensor_copy(tA[:], oA[:])
            nc.vector.tensor_copy(tB[:], oB[:])
            nc.sync.dma_start(AO[t][0:64, qs], tA[0:64, :])
            nc.sync.dma_start(AO[t][64:128, qs], tB[0:64, :])
            nc.sync.dma_start(denom[t:t + 1, qs], tA[64:65, :])
            nc.sync.dma_start(denom[t + 4:t + 5, qs], tB[64:65, :])

        # -- normalize this q-slab --
        nc.vector.reciprocal(denomR[:, qs], denom[:, qs])
        for t in range(4):
            bc = psW.tile([128, QTILE], F32, name="bc", tag="w")
            nc.tensor.matmul(bc[:], sel_t[:, t * 128:(t + 1) * 128],
                             denomR[:, qs], start=True, stop=True)
            nc.vector.tensor_mul(AO[t][:, qs], AO[t][:, qs], bc[:])

        # -- output projection for this q-slab --
        for jq in range(4 * j, 4 * j + 4):  # q-tiles of 128
            qsl = slice(jq * 128, (jq + 1) * 128)
            for dn in range(4):
                ps = psW.tile([128, QTILE], F32, name="psWo", tag="w")
                for fc in range(4):
                    nc.tensor.matmul(ps[:], _c(AO[fc][:, qsl]), _c(WO[fc, dn][:]),
                                     start=(fc == 0), stop=(fc == 3))
                og = evac.tile([128, QTILE], F32, name="og", tag="og")
                nc.vector.tensor_copy(og[:], ps[:])
                nc.sync.dma_start(
                    partial[jq * 128:(jq + 1) * 128,
                            dn * QTILE:(dn + 1) * QTILE], og[:])

        # -- reduce-scatter this slab across the 4 cores of the batch --
        if io.get("single"):
            nc.sync.dma_start(rs_out[j][:],
                              partial[j * QTILE:j * QTILE + 128, :])
        else:
            nc.gpsimd.collective_compute(
                "ReduceScatter",
                mybir.AluOpType.add,
                replica_groups=[[0, 1, 2, 3], [4, 5, 6, 7]],
                ins=[partial[j * QTILE:(j + 1) * QTILE, :]],
                outs=[rs_out[j][:]],
            )
        nc.sync.dma_start(out_full[j * 128:(j + 1) * 128, :], rs_out[j][:])

    psW.release()
    psS.release()
    evac.release()
    pexp.release()
    vap.release()
    qkv.release()
    aop.release()
    wop.release()
    dram.release()
    const.release()


def _build(single=False):
    nc = bacc.Bacc("TRN2", target_bir_lowering=False, debug=False,
                   num_devices=1 if single else NCORES)
    io = {
        "xT": nc.dram_tensor("xT", [D, S], F32R, kind="ExternalInput").ap(),
        "wq": nc.dram_tensor("wq", [D, FQ], F32R, kind="ExternalInput").ap(),
        "wkv": nc.dram_tensor("wkv", [D, FKV], F32R, kind="ExternalInput").ap(),
        "wo": nc.dram_tensor("wo", [FQ, D], F32R, kind="ExternalInput").ap(),
        "cos2": nc.dram_tensor("cos2", [128, S], F32, kind="ExternalInput").ap(),
        "sin2s": nc.dram_tensor("sin2s", [128, S], F32, kind="ExternalInput").ap(),
        "trimask": nc.dram_tensor("trimask", [KTILE, KTILE], F32,
                                  kind="ExternalInput").ap(),
        "sel": nc.dram_tensor("sel", [QH, FQ], F32, kind="ExternalInput").ap(),
        "out": nc.dram_tensor("out", [NSLAB * 128, D], F32, kind="ExternalOutput").ap(),
    }
    io["single"] = single
    with tile.TileContext(nc) as tc:
        _build_kernel(tc, io)
    nc.compile()
    return nc


_CACHE = {}


def _get_program():
    if "nc" not in _CACHE:
        _CACHE["nc"] = _build()
    return _CACHE["nc"]


def _host_inputs(x, wq, wk, wv, wo):
    x = np.ascontiguousarray(x, np.float32)
    inv = 1.0 / (10000.0 ** (np.arange(0, HD, 2, dtype=np.float64) / HD))
    pos = np.arange(S, dtype=np.float64)
    freqs = np.outer(pos, inv)  # [S, 32]
    emb = np.concatenate([freqs, freqs], axis=1)  # [S, 64]
    cos = np.cos(emb).T.astype(np.float32)  # [64, S]
    sin = np.sin(emb).T.astype(np.float32)
    cos2 = np.concatenate([cos, cos], axis=0)  # [128, S]
    sin2s = np.concatenate([-sin[:32], sin[32:], -sin[:32], sin[32:]], axis=0)

    kk, qq = np.meshgrid(np.arange(KTILE), np.arange(KTILE), indexing="ij")
    trimask = np.where(kk <= qq, 0.0, MASK_NEG).astype(np.float32)

    # attn_outT row layout per pair-tile t: rows 0:64 head t, 64:128 head t+4
    sel = np.zeros((QH, FQ), np.float32)
    for t in range(4):
        sel[t, t * 128:t * 128 + 64] = 1.0
        sel[t + 4, t * 128 + 64:(t + 1) * 128] = 1.0

    xT = [np.ascontiguousarray(x[b].T) for b in range(B)]
    in_maps = []
    for c in range(NCORES):
        b, g = c // NG, c % NG
        # pair-tile column order: heads (t, t+4) interleaved per 128-col tile
        qcols = []
        wrows = []
        for t in range(4):
            for h in (8 * g + t, 8 * g + t + 4):
                qcols.append(wq[:, h * HD:(h + 1) * HD])
                wrows.append(wo[h * HD:(h + 1) * HD, :])
        wq_p = np.ascontiguousarray(np.concatenate(qcols, axis=1), np.float32)
        wo_p = np.ascontiguousarray(np.concatenate(wrows, axis=0), np.float32)
        kv0 = 2 * g
        wkv_p = np.ascontiguousarray(np.concatenate(
            [wk[:, kv0 * HD:(kv0 + 2) * HD], wv[:, kv0 * HD:(kv0 + 2) * HD]],
            axis=1), np.float32)
        in_maps.append({
            "xT": xT[b], "wq": wq_p, "wkv": wkv_p, "wo": wo_p,
            "cos2": cos2, "sin2s": sin2s, "trimask": trimask, "sel": sel,
        })
    return in_maps


def run(x, wq, wk, wv, wo, trace=False, **trace_kwargs):
    nc = _get_program()
    in_maps = _host_inputs(x, wq, wk, wv, wo)
    res = run_bass_kernel_spmd(nc, in_maps, list(range(NCORES)),
                               trace=trace, **trace_kwargs)
    out = np.empty((B, S, D), np.float32)
    for b in range(B):
        for r in range(NG):
            shard = res.results[b * NG + r]["out"]  # [NSLAB*128, D]
            for j in range(NSLAB):
                out[b, j * QTILE + r * 128:j * QTILE + (r + 1) * 128, :] = \
                    shard[j * 128:(j + 1) * 128, :]
    return out, res


def kernel(x, wq, wk, wv, wo):
    out, _ = run(x, wq, wk, wv, wo)
    return out.astype(np.float32)
